# revision 18
# baseline (speedup 1.0000x reference)
"""NetVLAD pooling kernel for Trainium2 (Bass/Tile), 8-core data-parallel.

Reference computation (per batch b):
    scores = conv_w @ x[b]                  # [K, N]
    assign = softmax(scores, axis=K)
    vlad   = x[b] @ assign.T - centers * assign.sum(n)   # [D, K]
    vlad  /= max(||vlad||_2 over D, eps)    # intra-norm per cluster column
    desc   = vlad.reshape(D*K) / max(||.||_2, eps)

Shapes: x [32, 512, 1024] f32, conv_w [64, 512], centers [512, 64],
output desc [32, 32768] f32.  Sharding: data-parallel over batch,
4 batches per core; params replicated.

v6 design (bf16 PE path; this kernel is PE-bound and every matmul pays
its own serial LDWEIGHTS in this toolchain, so the structure minimizes
weight-load columns and instruction count):

  * x is cast f32->bf16 *during* the DMA (SWDGE on gpsimd) in 8
    half-batch chunks so compute pipelines behind the load.
  * scores run in natural [K,N] layout with conv_w^T stationary (64-col
    weight loads) streaming x at 512 columns per matmul; the softmax
    exp happens in that layout on ACT, and the small exp'd scores are
    then PE-transposed per 128-chunk into [n,k] (8 transposes/batch)
    where the k-reduce is a cheap free-dim DVE reduce.
  * x itself is PE-transposed chunkwise (32 transpose matmuls/batch,
    bf16 PSUM) - the unavoidable cost of the n-contraction in vlad.
  * softmax reciprocal folds into AN = ET*rec per chunk (DVE), so the
    PSUM->SBUF moves of xT are plain bf16 copies split DVE/ACT, and the
    assign row-sums are ones-matmuls sharing AN as stationary.
  * per-j software pipelining: chunk j's softmax/copy chain runs under
    chunk j+1's PE matmuls; vlad accumulates per chunk right behind.
  * batches pair up in PSUM ([0:64] even batch, [64:128] odd batch via
    matmul column tiling) so the epilogue's heavy [*,512] ops run per
    batch as soon as that batch's vlad closes, while the cheap scalar
    chain (one sqrt per pair keeps the ACT table from thrashing between
    the exp and sqrt function sets), the output transposes and stores
    run at pair level.
  * the second L2 normalization is folded to 1/8 (each of the K=64 unit
    columns contributes 1 to ||desc||^2, so ||desc|| = 8).

bf16 rounding of x/w/assign contributes ~2e-3 relative error, well
inside the 2e-2 gate (measured: see test.py output).
"""

import numpy as np

import concourse.bass as bass
from concourse import bacc
import concourse.mybir as mybir
import concourse.tile as tile
from concourse.bass_utils import run_bass_kernel_spmd
from concourse.masks import make_identity

B, D, K, N = 32, 512, 64, 1024
NCORES = 8
BC = B // NCORES          # batches per core
F32 = mybir.dt.float32
BF16 = mybir.dt.bfloat16
EPS = 1e-12

DC = D // 128             # d chunks (4)
NB = N // 128             # n chunks per batch (8)
NHJ = NB // 2             # n chunks per half (4)


def _netvlad_core(ctx, tc, out, x, w, c):
    """Emit the per-core tile program.

    out: desc [BC, D*K] f32 DRAM     x: [BC, D, N] f32 DRAM
    w:   conv_w [K, D] f32 DRAM      c: centers [D, K] f32 DRAM
    """
    nc = tc.nc
    Exp = mybir.ActivationFunctionType.Exp
    Square = mybir.ActivationFunctionType.Square

    const = ctx.enter_context(tc.tile_pool(name="const", bufs=1))
    xpool = ctx.enter_context(tc.tile_pool(name="xp", bufs=1))
    epool = ctx.enter_context(tc.tile_pool(name="ep", bufs=2))
    atp = ctx.enter_context(tc.tile_pool(name="atp", bufs=2))
    sp = ctx.enter_context(tc.tile_pool(name="sp", bufs=2))
    xst = ctx.enter_context(tc.tile_pool(name="xst", bufs=4))
    vp = ctx.enter_context(tc.tile_pool(name="vp", bufs=2))
    op = ctx.enter_context(tc.tile_pool(name="op", bufs=2))
    # PSUM: sc(1) + et(1) + xt(2) + v(2) + o(1) + as(1) = 8 banks
    ps_sc = ctx.enter_context(tc.tile_pool(name="ps_sc", bufs=1, space="PSUM"))
    ps_et = ctx.enter_context(tc.tile_pool(name="ps_et", bufs=1, space="PSUM"))
    ps_xt = ctx.enter_context(tc.tile_pool(name="ps_xt", bufs=2, space="PSUM"))
    ps_v = ctx.enter_context(tc.tile_pool(name="ps_v", bufs=2, space="PSUM"))
    ps_o = ctx.enter_context(tc.tile_pool(name="ps_o", bufs=1, space="PSUM"))
    ps_as = ctx.enter_context(tc.tile_pool(name="ps_as", bufs=1, space="PSUM"))

    # ---- startup-critical ordering.  SWDGE descriptor emission is ~3us
    # per half-batch and strictly serial on Q7, so batch 0's first half
    # leads; w/c ride HWDGE on the idle sync engine instead.
    xb = []
    xsrcs = []
    for b in range(BC):
        xt_ = xpool.tile([128, DC, N], BF16, tag="x", name=f"x{b}", bufs=BC)
        xb.append(xt_)
        xsrcs.append(x[b].rearrange("(cc p) n -> p cc n", p=128))
    h0 = slice(0, 512)
    nc.gpsimd.dma_start(xb[0][:, :, h0], xsrcs[0][:, :, h0])
    identb = const.tile([128, 128], BF16, tag="identb")
    make_identity(nc, identb)
    h1 = slice(512, 1024)
    nc.gpsimd.dma_start(xb[0][:, :, h1], xsrcs[0][:, :, h1])
    for b in range(1, BC):
        for h in range(2):
            ns = slice(h * 512, (h + 1) * 512)
            nc.gpsimd.dma_start(xb[b][:, :, ns], xsrcs[b][:, :, ns])

    wn = const.tile([K, D], F32, tag="wn")
    nc.sync.dma_start(wn, w)
    wnb = const.tile([K, D], BF16, tag="wnb")
    nc.vector.tensor_copy(wnb, wn)
    cnat = const.tile([128, DC, K], F32, tag="cnat")
    nc.sync.dma_start(cnat, c.rearrange("(cc p) k -> p cc k", p=128))
    cnb = const.tile([128, DC, K], BF16, tag="cnb")
    nc.vector.tensor_copy(cnb, cnat)

    # conv_w^T in bf16: wTb [128(d), 4, 64(k)]
    wT_ps = ps_xt.tile([128, DC, K], BF16, tag="xt", name="wT_ps")
    for cc in range(DC):
        nc.tensor.transpose(
            wT_ps[:, cc, :], wnb[:, cc * 128:(cc + 1) * 128], identb[:K, :K]
        )
    wTb = const.tile([128, DC, K], BF16, tag="wTb")
    nc.vector.tensor_copy(wTb, wT_ps)
    onesb = const.tile([128, 2], BF16, tag="onesb")
    nc.vector.memset(onesb, 1.0)

    # centers^T replicated on both partition halves: cT2 [128(k2), 512(d)]
    # (regular bf16 matmuls: walrus requires transpose-MM outputs at PSUM
    # partition 0, and half=1 lands at partition 64)
    cT2_ps = ps_o.tile([128, DC, 128], F32, tag="o", name="cT2_ps")
    for half in range(2):
        for cc in range(DC):
            nc.tensor.matmul(
                cT2_ps[64 * half:64 * half + 64, cc, :],
                lhsT=cnb[:, cc, :],
                rhs=identb,
            )
    cT2 = const.tile([128, DC, 128], F32, tag="cT2")
    nc.scalar.copy(cT2, cT2_ps)
    cT2f = cT2.rearrange("p cc d -> p (cc d)")

    # assign row-sum accumulators for all 4 batches in one PSUM bank:
    # batch b -> partitions 64*(b%2).., cols 2*(b//2)..
    as_t = ps_as.tile([128, 2 * (BC // 2)], F32, tag="as", name="as_t")

    desc_v = out.rearrange(
        "(bp b2) (cc p k) -> p cc bp b2 k", b2=2, cc=DC, p=128, k=K
    )

    # ---- per batch ----------------------------------------------------
    v2_ps = None
    Vpair = sspair = None
    for b in range(BC):
        bp, b2 = b // 2, b % 2
        base = 64 * b2
        if b2 == 0:
            v2_ps = ps_v.tile([128, 512], F32, tag="v", name=f"v{bp}")
            Vpair = vp.tile([128, 512], F32, tag="V", name=f"V{bp}")
            sspair = sp.tile([128, 1], F32, tag="ss", name=f"ss{bp}")

        E = epool.tile([K, N], BF16, tag="E", name=f"E{b}")
        ET = ps_et.tile([128, NB, K], BF16, tag="et", name=f"ET{b}")
        AN = atp.tile([128, NB, K], BF16, tag="AN", name=f"AN{b}")
        red = sp.tile([128, NB], F32, tag="red", name=f"red{b}")
        rec = sp.tile([128, NB], F32, tag="rec", name=f"rec{b}")

        for h in range(2):
            ns = slice(h * 512, (h + 1) * 512)
            # scores [k, n-half], conv_w^T stationary (64-col LDW)
            sc = ps_sc.tile([K, 512], F32, tag="sc", name=f"sc{b}_{h}")
            for cc in range(DC):
                nc.tensor.matmul(
                    sc,
                    lhsT=wTb[:, cc, :],
                    rhs=xb[b][:, cc, ns],
                    start=(cc == 0),
                    stop=(cc == DC - 1),
                )
            # exp in natural layout (no max-subtraction: scores ~N(0,1)
            # since conv_w is scaled 1/sqrt(D); exp cannot overflow)
            nc.scalar.activation(E[:, ns], sc, func=Exp)

            # x^T chunks for this half (independent of the softmax chain,
            # keeps the PE busy while ACT computes the exp)
            xt_ps_h = []
            for j in range(NHJ * h, NHJ * h + NHJ):
                xt_ps = ps_xt.tile(
                    [128, DC, 128], BF16, tag="xt", name=f"xt{b}_{j}"
                )
                for cc in range(DC):
                    nc.tensor.transpose(
                        xt_ps[:, cc, :],
                        xb[b][:, cc, j * 128:(j + 1) * 128],
                        identb,
                    )
                xt_ps_h.append(xt_ps)

            for jj, j in enumerate(range(NHJ * h, NHJ * h + NHJ)):
                # E^T chunk [n, k] via PE (small: 64-col identity stream)
                nc.tensor.transpose(
                    ET[:, j, :], E[:, j * 128:(j + 1) * 128], identb[:K, :K]
                )
                # softmax normalization for chunk j
                nc.vector.tensor_reduce(
                    red[:, j:j + 1], ET[:, j, :], axis=mybir.AxisListType.X,
                    op=mybir.AluOpType.add,
                )
                nc.vector.reciprocal(rec[:, j:j + 1], red[:, j:j + 1])
                nc.vector.tensor_scalar(
                    AN[:, j, :], ET[:, j, :], rec[:, j:j + 1], None,
                    op0=mybir.AluOpType.mult,
                )
                # xT to SBUF as a plain bf16 copy (normalization is in AN)
                xsT = xst.tile(
                    [128, DC, 128], BF16, tag="xs", name=f"xs{b}_{j}", bufs=4
                )
                xs_flat = xsT.rearrange("p cc d -> p (cc d)")
                xt_flat = xt_ps_h[jj].rearrange("p cc d -> p (cc d)")
                if j % 2 == 0:
                    nc.vector.tensor_copy(xs_flat, xt_flat)
                else:
                    nc.scalar.copy(xs_flat, xt_flat)
                # vladT [k,d] accumulated over n chunks; odd batch goes to
                # PSUM partitions 64-127 via column tiling
                nc.tensor.matmul(
                    v2_ps[base:base + 64, :],
                    lhsT=AN[:, j, :],
                    rhs=xs_flat,
                    start=(j == 0),
                    stop=(j == NB - 1),
                )
                # assign row sums: sum_n AN[n,k]
                nc.tensor.matmul(
                    as_t[base:base + 64, 2 * bp:2 * bp + 2],
                    lhsT=AN[:, j, :],
                    rhs=onesb,
                    start=(j == 0),
                    stop=(j == NB - 1),
                )

        # ---- per-batch epilogue (heavy [*,512] ops start as soon as this
        # batch's vlad closes; partition range matches the PSUM half) ----
        asum = sp.tile([128, 1], F32, tag="asum", name=f"asum{bp}_{b2}")
        nc.scalar.mul(
            asum[base:base + 64], as_t[base:base + 64, 2 * bp:2 * bp + 1],
            -1.0,
        )
        nc.vector.scalar_tensor_tensor(
            Vpair[base:base + 64, :], cT2f[base:base + 64, :],
            asum[base:base + 64], v2_ps[base:base + 64, :],
            op0=mybir.AluOpType.mult, op1=mybir.AluOpType.add,
        )
        sq = vp.tile([128, 512], F32, tag="sq", name=f"sq{bp}_{b2}")
        nc.scalar.activation(
            sq[base:base + 64, :], Vpair[base:base + 64, :], func=Square,
            accum_out=sspair[base:base + 64],
        )

        if b2 == 1:
            # ---- pair epilogue: one sqrt visit per pair (ACT table set
            # switches exp<->sqrt cost 1.3us each), transpose, store ----
            nrm = sp.tile([128, 1], F32, tag="nrm", name=f"nrm{bp}")
            nc.scalar.sqrt(nrm, sspair)
            nrmc = sp.tile([128, 1], F32, tag="nrmc", name=f"nrmc{bp}")
            nc.vector.tensor_scalar_max(nrmc, nrm, EPS)
            rinv = sp.tile([128, 1], F32, tag="rinv", name=f"rinv{bp}")
            nc.vector.reciprocal(rinv, nrmc)
            Vn = vp.tile([128, 512], BF16, tag="Vn", name=f"Vn{bp}")
            nc.vector.tensor_scalar(
                Vn, Vpair, rinv, 1.0 / 8.0,
                op0=mybir.AluOpType.mult, op1=mybir.AluOpType.mult,
            )

            # transpose [k2, d] -> [d, k2] and store both batches
            o_ps = ps_o.tile([128, DC, 128], BF16, tag="o", name=f"o{bp}")
            for cc in range(DC):
                nc.tensor.transpose(
                    o_ps[:, cc, :], Vn[:, cc * 128:(cc + 1) * 128], identb
                )
            o_sb = op.tile([128, DC, 128], F32, tag="osb", name=f"osb{bp}")
            nc.scalar.copy(o_sb, o_ps)
            for b2o in range(2):
                nc.sync.dma_start(
                    desc_v[:, :, bp, b2o, :],
                    o_sb[:, :, b2o * K:(b2o + 1) * K],
                )


_NC_CACHE = None


def _build_nc():
    global _NC_CACHE
    if _NC_CACHE is not None:
        return _NC_CACHE
    from contextlib import ExitStack

    nc = bacc.Bacc("TRN2", target_bir_lowering=False, debug=False,
                   num_devices=NCORES)
    x = nc.dram_tensor("x", [BC, D, N], F32, kind="ExternalInput").ap()
    w = nc.dram_tensor("conv_w", [K, D], F32, kind="ExternalInput").ap()
    c = nc.dram_tensor("centers", [D, K], F32, kind="ExternalInput").ap()
    out = nc.dram_tensor("desc", [BC, D * K], F32, kind="ExternalOutput").ap()
    with tile.TileContext(nc) as tc, ExitStack() as ctx:
        _netvlad_core(ctx, tc, out, x, w, c)
    nc.compile()
    _NC_CACHE = nc
    return nc


def kernel(x, conv_w, centers):
    x = np.ascontiguousarray(x, dtype=np.float32)
    conv_w = np.ascontiguousarray(conv_w, dtype=np.float32)
    centers = np.ascontiguousarray(centers, dtype=np.float32)
    nc = _build_nc()
    in_maps = [
        {
            "x": np.ascontiguousarray(x[i * BC:(i + 1) * BC]),
            "conv_w": conv_w,
            "centers": centers,
        }
        for i in range(NCORES)
    ]
    res = run_bass_kernel_spmd(nc, in_maps, core_ids=list(range(NCORES)))
    return np.concatenate([r["desc"] for r in res.results], axis=0)


# revision 19
# speedup vs baseline: 1.1113x; 1.1113x over previous
"""NetVLAD pooling kernel for Trainium2 (Bass/Tile), 8-core data-parallel.

Reference computation (per batch b):
    scores = conv_w @ x[b]                  # [K, N]
    assign = softmax(scores, axis=K)
    vlad   = x[b] @ assign.T - centers * assign.sum(n)   # [D, K]
    vlad  /= max(||vlad||_2 over D, eps)    # intra-norm per cluster column
    desc   = vlad.reshape(D*K) / max(||.||_2, eps)

Shapes: x [32, 512, 1024] f32, conv_w [64, 512], centers [512, 64],
output desc [32, 32768] f32.  Sharding: data-parallel over batch,
4 batches per core; params replicated.

v6 design (bf16 PE path; this kernel is PE-bound and every matmul pays
its own serial LDWEIGHTS in this toolchain, so the structure minimizes
weight-load columns and instruction count):

  * x is cast f32->bf16 *during* the DMA (SWDGE on gpsimd) in 8
    half-batch chunks so compute pipelines behind the load.
  * scores run in natural [K,N] layout with conv_w^T stationary (64-col
    weight loads) streaming x at 512 columns per matmul; the softmax
    exp happens in that layout on ACT, and the small exp'd scores are
    then PE-transposed per 128-chunk into [n,k] (8 transposes/batch)
    where the k-reduce is a cheap free-dim DVE reduce.
  * x itself is PE-transposed chunkwise (32 transpose matmuls/batch,
    bf16 PSUM) - the unavoidable cost of the n-contraction in vlad.
  * softmax reciprocal folds into AN = ET*rec per chunk (DVE), so the
    PSUM->SBUF moves of xT are plain bf16 copies split DVE/ACT, and the
    assign row-sums are ones-matmuls sharing AN as stationary.
  * per-j software pipelining: chunk j's softmax/copy chain runs under
    chunk j+1's PE matmuls; vlad accumulates per chunk right behind.
  * batches pair up in PSUM ([0:64] even batch, [64:128] odd batch via
    matmul column tiling) so the epilogue's heavy [*,512] ops run per
    batch as soon as that batch's vlad closes, while the cheap scalar
    chain (one sqrt per pair keeps the ACT table from thrashing between
    the exp and sqrt function sets), the output transposes and stores
    run at pair level.
  * the second L2 normalization is folded to 1/8 (each of the K=64 unit
    columns contributes 1 to ||desc||^2, so ||desc|| = 8).

bf16 rounding of x/w/assign contributes ~2e-3 relative error, well
inside the 2e-2 gate (measured: see test.py output).
"""

import numpy as np

import concourse.bass as bass
from concourse import bacc
import concourse.mybir as mybir
import concourse.tile as tile
from concourse.bass_utils import run_bass_kernel_spmd
from concourse.masks import make_identity

B, D, K, N = 32, 512, 64, 1024
NCORES = 8
BC = B // NCORES          # batches per core
F32 = mybir.dt.float32
BF16 = mybir.dt.bfloat16
EPS = 1e-12

DC = D // 128             # d chunks (4)
NB = N // 128             # n chunks per batch (8)
NHJ = NB // 2             # n chunks per half (4)


def _netvlad_core(ctx, tc, out, x, w, c):
    """Emit the per-core tile program.

    out: desc [BC, D*K] f32 DRAM     x: [BC, D, N] f32 DRAM
    w:   conv_w [K, D] f32 DRAM      c: centers [D, K] f32 DRAM
    """
    nc = tc.nc
    Exp = mybir.ActivationFunctionType.Exp
    Square = mybir.ActivationFunctionType.Square

    const = ctx.enter_context(tc.tile_pool(name="const", bufs=1))
    xpool = ctx.enter_context(tc.tile_pool(name="xp", bufs=1))
    epool = ctx.enter_context(tc.tile_pool(name="ep", bufs=2))
    atp = ctx.enter_context(tc.tile_pool(name="atp", bufs=2))
    sp = ctx.enter_context(tc.tile_pool(name="sp", bufs=2))
    xst = ctx.enter_context(tc.tile_pool(name="xst", bufs=4))
    vp = ctx.enter_context(tc.tile_pool(name="vp", bufs=2))
    op = ctx.enter_context(tc.tile_pool(name="op", bufs=2))
    # PSUM: sc(1) + et(1) + xt(2) + v(2) + o(1) + as(1) = 8 banks
    ps_sc = ctx.enter_context(tc.tile_pool(name="ps_sc", bufs=1, space="PSUM"))
    ps_et = ctx.enter_context(tc.tile_pool(name="ps_et", bufs=1, space="PSUM"))
    ps_xt = ctx.enter_context(tc.tile_pool(name="ps_xt", bufs=2, space="PSUM"))
    ps_v = ctx.enter_context(tc.tile_pool(name="ps_v", bufs=2, space="PSUM"))
    ps_o = ctx.enter_context(tc.tile_pool(name="ps_o", bufs=1, space="PSUM"))
    ps_as = ctx.enter_context(tc.tile_pool(name="ps_as", bufs=1, space="PSUM"))

    # ---- startup-critical ordering.  SWDGE descriptor emission is ~3us
    # per half-batch and strictly serial on Q7, so batch 0's first half
    # leads; w/c ride HWDGE on the idle sync engine instead.
    xb = []
    xsrcs = []
    for b in range(BC):
        xt_ = xpool.tile([128, DC, N], BF16, tag="x", name=f"x{b}", bufs=BC)
        xb.append(xt_)
        xsrcs.append(x[b].rearrange("(cc p) n -> p cc n", p=128))
    h0 = slice(0, 512)
    nc.gpsimd.dma_start(xb[0][:, :, h0], xsrcs[0][:, :, h0])
    identb = const.tile([128, 128], BF16, tag="identb")
    make_identity(nc, identb)
    h1 = slice(512, 1024)
    nc.gpsimd.dma_start(xb[0][:, :, h1], xsrcs[0][:, :, h1])
    for b in range(1, BC):
        for h in range(2):
            ns = slice(h * 512, (h + 1) * 512)
            nc.gpsimd.dma_start(xb[b][:, :, ns], xsrcs[b][:, :, ns])

    wn = const.tile([K, D], F32, tag="wn")
    nc.sync.dma_start(wn, w)
    wnb = const.tile([K, D], BF16, tag="wnb")
    nc.vector.tensor_copy(wnb, wn)
    cnat = const.tile([128, DC, K], F32, tag="cnat")
    nc.sync.dma_start(cnat, c.rearrange("(cc p) k -> p cc k", p=128))
    cnb = const.tile([128, DC, K], BF16, tag="cnb")
    nc.vector.tensor_copy(cnb, cnat)

    # conv_w^T in bf16: wTb [128(d), 4, 64(k)]
    wT_ps = ps_xt.tile([128, DC, K], BF16, tag="xt", name="wT_ps")
    for cc in range(DC):
        nc.tensor.transpose(
            wT_ps[:, cc, :], wnb[:, cc * 128:(cc + 1) * 128], identb[:K, :K]
        )
    wTb = const.tile([128, DC, K], BF16, tag="wTb")
    nc.vector.tensor_copy(wTb, wT_ps)
    onesb = const.tile([128, 2], BF16, tag="onesb")
    nc.vector.memset(onesb, 1.0)

    # centers^T replicated on both partition halves: cT2 [128(k2), 512(d)]
    # (regular bf16 matmuls: walrus requires transpose-MM outputs at PSUM
    # partition 0, and half=1 lands at partition 64)
    cT2_ps = ps_o.tile([128, DC, 128], F32, tag="o", name="cT2_ps")
    for half in range(2):
        for cc in range(DC):
            nc.tensor.matmul(
                cT2_ps[64 * half:64 * half + 64, cc, :],
                lhsT=cnb[:, cc, :],
                rhs=identb,
            )
    cT2 = const.tile([128, DC, 128], F32, tag="cT2")
    nc.scalar.copy(cT2, cT2_ps)
    cT2f = cT2.rearrange("p cc d -> p (cc d)")

    # assign row-sum accumulators for all 4 batches in one PSUM bank:
    # batch b -> partitions 64*(b%2).., cols 2*(b//2)..
    as_t = ps_as.tile([128, 2 * (BC // 2)], F32, tag="as", name="as_t")

    desc_v = out.rearrange(
        "(bp b2) (cc p k) -> p cc bp b2 k", b2=2, cc=DC, p=128, k=K
    )

    # ---- per batch ----------------------------------------------------
    v2_ps = None
    Vpair = sspair = None
    for b in range(BC):
        bp, b2 = b // 2, b % 2
        base = 64 * b2
        if b2 == 0:
            v2_ps = ps_v.tile([128, 512], F32, tag="v", name=f"v{bp}")
            Vpair = vp.tile([128, 512], F32, tag="V", name=f"V{bp}")
            sspair = sp.tile([128, 1], F32, tag="ss", name=f"ss{bp}")

        E = epool.tile([K, N], BF16, tag="E", name=f"E{b}")
        ET = ps_et.tile([128, NB, K], BF16, tag="et", name=f"ET{b}")
        AN = atp.tile([128, NB, K], BF16, tag="AN", name=f"AN{b}")
        red = sp.tile([128, NB], F32, tag="red", name=f"red{b}")
        rec = sp.tile([128, NB], F32, tag="rec", name=f"rec{b}")

        # software-pipelined emission: the PE queue is strict FIFO, so each
        # chunk's vlad/asum matmuls are emitted one chunk late - the next
        # chunk's x-transposes sit between ET[j] and vlad[j] in the queue
        # and absorb the DVE softmax-chain latency.
        pending = None

        def emit_vlad(j):
            xs_flat, first, last = pending_info[j]
            nc.tensor.matmul(
                v2_ps[base:base + 64, :],
                lhsT=AN[:, j, :],
                rhs=xs_flat,
                start=first,
                stop=last,
            )
            nc.tensor.matmul(
                as_t[base:base + 64, 2 * bp:2 * bp + 2],
                lhsT=AN[:, j, :],
                rhs=onesb,
                start=first,
                stop=last,
            )

        pending_info = {}
        for h in range(2):
            ns = slice(h * 512, (h + 1) * 512)
            # scores [k, n-half], conv_w^T stationary (64-col LDW)
            sc = ps_sc.tile([K, 512], F32, tag="sc", name=f"sc{b}_{h}")
            for cc in range(DC):
                nc.tensor.matmul(
                    sc,
                    lhsT=wTb[:, cc, :],
                    rhs=xb[b][:, cc, ns],
                    start=(cc == 0),
                    stop=(cc == DC - 1),
                )
            # exp in natural layout (no max-subtraction: scores ~N(0,1)
            # since conv_w is scaled 1/sqrt(D); exp cannot overflow)
            nc.scalar.activation(E[:, ns], sc, func=Exp)

            for j in range(NHJ * h, NHJ * h + NHJ):
                # x^T chunk via PE transpose-mode (bf16 PSUM)
                xt_ps = ps_xt.tile(
                    [128, DC, 128], BF16, tag="xt", name=f"xt{b}_{j}"
                )
                for cc in range(DC):
                    nc.tensor.transpose(
                        xt_ps[:, cc, :],
                        xb[b][:, cc, j * 128:(j + 1) * 128],
                        identb,
                    )
                # E^T chunk [n, k] via PE (small: 64-col identity stream)
                nc.tensor.transpose(
                    ET[:, j, :], E[:, j * 128:(j + 1) * 128], identb[:K, :K]
                )
                if pending is not None:
                    emit_vlad(pending)
                # softmax normalization for chunk j
                nc.vector.tensor_reduce(
                    red[:, j:j + 1], ET[:, j, :], axis=mybir.AxisListType.X,
                    op=mybir.AluOpType.add,
                )
                nc.vector.reciprocal(rec[:, j:j + 1], red[:, j:j + 1])
                nc.vector.tensor_scalar(
                    AN[:, j, :], ET[:, j, :], rec[:, j:j + 1], None,
                    op0=mybir.AluOpType.mult,
                )
                # xT to SBUF as a plain bf16 copy (normalization is in AN)
                xsT = xst.tile(
                    [128, DC, 128], BF16, tag="xs", name=f"xs{b}_{j}", bufs=4
                )
                xs_flat = xsT.rearrange("p cc d -> p (cc d)")
                xt_flat = xt_ps.rearrange("p cc d -> p (cc d)")
                if j % 2 == 0:
                    nc.vector.tensor_copy(xs_flat, xt_flat)
                else:
                    nc.scalar.copy(xs_flat, xt_flat)
                pending_info[j] = (xs_flat, j == 0, j == NB - 1)
                pending = j
        emit_vlad(pending)

        # ---- per-batch epilogue (heavy [*,512] ops start as soon as this
        # batch's vlad closes; partition range matches the PSUM half) ----
        asum = sp.tile([128, 1], F32, tag="asum", name=f"asum{bp}_{b2}")
        nc.scalar.mul(
            asum[base:base + 64], as_t[base:base + 64, 2 * bp:2 * bp + 1],
            -1.0,
        )
        nc.vector.scalar_tensor_tensor(
            Vpair[base:base + 64, :], cT2f[base:base + 64, :],
            asum[base:base + 64], v2_ps[base:base + 64, :],
            op0=mybir.AluOpType.mult, op1=mybir.AluOpType.add,
        )
        sq = vp.tile([128, 512], F32, tag="sq", name=f"sq{bp}_{b2}")
        nc.scalar.activation(
            sq[base:base + 64, :], Vpair[base:base + 64, :], func=Square,
            accum_out=sspair[base:base + 64],
        )

        if b2 == 1:
            # ---- pair epilogue: one sqrt visit per pair (ACT table set
            # switches exp<->sqrt cost 1.3us each), transpose, store ----
            nrm = sp.tile([128, 1], F32, tag="nrm", name=f"nrm{bp}")
            nc.scalar.sqrt(nrm, sspair)
            nrmc = sp.tile([128, 1], F32, tag="nrmc", name=f"nrmc{bp}")
            nc.vector.tensor_scalar_max(nrmc, nrm, EPS)
            rinv = sp.tile([128, 1], F32, tag="rinv", name=f"rinv{bp}")
            nc.vector.reciprocal(rinv, nrmc)
            Vn = vp.tile([128, 512], BF16, tag="Vn", name=f"Vn{bp}")
            nc.vector.tensor_scalar(
                Vn, Vpair, rinv, 1.0 / 8.0,
                op0=mybir.AluOpType.mult, op1=mybir.AluOpType.mult,
            )

            # transpose [k2, d] -> [d, k2] and store both batches
            o_ps = ps_o.tile([128, DC, 128], BF16, tag="o", name=f"o{bp}")
            for cc in range(DC):
                nc.tensor.transpose(
                    o_ps[:, cc, :], Vn[:, cc * 128:(cc + 1) * 128], identb
                )
            o_sb = op.tile([128, DC, 128], F32, tag="osb", name=f"osb{bp}")
            nc.scalar.copy(o_sb, o_ps)
            for b2o in range(2):
                nc.sync.dma_start(
                    desc_v[:, :, bp, b2o, :],
                    o_sb[:, :, b2o * K:(b2o + 1) * K],
                )


_NC_CACHE = None


def _build_nc():
    global _NC_CACHE
    if _NC_CACHE is not None:
        return _NC_CACHE
    from contextlib import ExitStack

    nc = bacc.Bacc("TRN2", target_bir_lowering=False, debug=False,
                   num_devices=NCORES)
    x = nc.dram_tensor("x", [BC, D, N], F32, kind="ExternalInput").ap()
    w = nc.dram_tensor("conv_w", [K, D], F32, kind="ExternalInput").ap()
    c = nc.dram_tensor("centers", [D, K], F32, kind="ExternalInput").ap()
    out = nc.dram_tensor("desc", [BC, D * K], F32, kind="ExternalOutput").ap()
    with tile.TileContext(nc) as tc, ExitStack() as ctx:
        _netvlad_core(ctx, tc, out, x, w, c)
    nc.compile()
    _NC_CACHE = nc
    return nc


def kernel(x, conv_w, centers):
    x = np.ascontiguousarray(x, dtype=np.float32)
    conv_w = np.ascontiguousarray(conv_w, dtype=np.float32)
    centers = np.ascontiguousarray(centers, dtype=np.float32)
    nc = _build_nc()
    in_maps = [
        {
            "x": np.ascontiguousarray(x[i * BC:(i + 1) * BC]),
            "conv_w": conv_w,
            "centers": centers,
        }
        for i in range(NCORES)
    ]
    res = run_bass_kernel_spmd(nc, in_maps, core_ids=list(range(NCORES)))
    return np.concatenate([r["desc"] for r in res.results], axis=0)


# revision 20
# speedup vs baseline: 1.3435x; 1.2089x over previous
"""NetVLAD pooling kernel for Trainium2 (Bass/Tile), 8-core data-parallel.

Reference computation (per batch b):
    scores = conv_w @ x[b]                  # [K, N]
    assign = softmax(scores, axis=K)
    vlad   = x[b] @ assign.T - centers * assign.sum(n)   # [D, K]
    vlad  /= max(||vlad||_2 over D, eps)    # intra-norm per cluster column
    desc   = vlad.reshape(D*K) / max(||.||_2, eps)

Shapes: x [32, 512, 1024] f32, conv_w [64, 512], centers [512, 64],
output desc [32, 32768] f32.  Sharding: data-parallel over batch,
4 batches per core; params replicated.

v8 design notes.  The kernel is PE-bound; in this toolchain every
matmul self-loads its stationary operand (~115ns serial LDWEIGHTS,
no dedup) and transpose-mode matmuls run on a cold clock (~250ns), so
the structure minimizes PE instruction count and keeps the hot loop on
regular (HAM-warming) matmuls:

  * x casts f32->bf16 *during* the DMA (SWDGE, gpsimd) in 8 half-batch
    chunks; batch 0's first half leads the Q7 queue; conv_w/centers ride
    HWDGE on the idle sync engine.
  * scoresT [n,k] come from x-chunk-stationary matmuls streaming
    conv_w^T (the same stationary then streams the identity to emit
    xT [n,d]) - softmax over k is then a free-dim DVE reduce, no E
    transposes, and x is never re-loaded from another layout.
  * the xT identity-matmuls are REGULAR matmuls into f32 PSUM; the
    PSUM->SBUF bf16 copies alternate DVE/ACT per chunk.
  * softmax chain runs per half-batch (8 wide ops, not 32 narrow ones);
    normalization folds into AN = exp*rec, so copies stay plain and the
    assign row-sums are ones-matmuls off the same AN stationaries.
  * vlad/asum matmuls are emitted one half-batch late (PE queue is
    strict FIFO): the next half's score/transpose matmuls sit between
    them and the softmax chain they depend on, so the PE never stalls.
  * batches pair into PSUM halves ([0:64] even, [64:128] odd, matmul
    column tiling); the epilogue's wide ops run per batch as soon as its
    vlad closes, the cheap scalar chain runs once per pair (single sqrt
    visit per pair - the ACT table switch exp<->sqrt costs 1.3us), and
    the final [k,d]->[d,k] flip uses regular identity matmuls.
  * the second L2 normalization folds to 1/8 (the K=64 unit columns give
    ||desc|| = 8 exactly).

bf16 rounding of x/w/assign contributes ~2e-3 relative error, well
inside the 2e-2 gate (measured: see test.py output).
"""

import numpy as np

import concourse.bass as bass
from concourse import bacc
import concourse.mybir as mybir
import concourse.tile as tile
from concourse.bass_utils import run_bass_kernel_spmd
from concourse.masks import make_identity

B, D, K, N = 32, 512, 64, 1024
NCORES = 8
BC = B // NCORES          # batches per core
F32 = mybir.dt.float32
BF16 = mybir.dt.bfloat16
EPS = 1e-12

DC = D // 128             # d chunks (4)
NB = N // 128             # n chunks per batch (8)
NHJ = NB // 2             # n chunks per half (4)


def _netvlad_core(ctx, tc, out, x, w, c):
    """Emit the per-core tile program.

    out: desc [BC, D*K] f32 DRAM     x: [BC, D, N] f32 DRAM
    w:   conv_w [K, D] f32 DRAM      c: centers [D, K] f32 DRAM
    """
    nc = tc.nc
    Exp = mybir.ActivationFunctionType.Exp
    Square = mybir.ActivationFunctionType.Square

    const = ctx.enter_context(tc.tile_pool(name="const", bufs=1))
    xpool = ctx.enter_context(tc.tile_pool(name="xp", bufs=1))
    atp = ctx.enter_context(tc.tile_pool(name="atp", bufs=2))
    sp = ctx.enter_context(tc.tile_pool(name="sp", bufs=2))
    xst = ctx.enter_context(tc.tile_pool(name="xst", bufs=8))
    vp = ctx.enter_context(tc.tile_pool(name="vp", bufs=2))
    op = ctx.enter_context(tc.tile_pool(name="op", bufs=2))
    # PSUM: st(2) + xt(2) + v(2) + o(1) + as(1) = 8 banks
    ps_st = ctx.enter_context(tc.tile_pool(name="ps_st", bufs=2, space="PSUM"))
    ps_xt = ctx.enter_context(tc.tile_pool(name="ps_xt", bufs=2, space="PSUM"))
    ps_v = ctx.enter_context(tc.tile_pool(name="ps_v", bufs=2, space="PSUM"))
    ps_o = ctx.enter_context(tc.tile_pool(name="ps_o", bufs=1, space="PSUM"))
    ps_as = ctx.enter_context(tc.tile_pool(name="ps_as", bufs=1, space="PSUM"))

    # ---- startup-critical ordering.  SWDGE descriptor emission is ~3us
    # per half-batch and strictly serial on Q7, so batch 0's first half
    # leads; identity build interleaves; w/c ride HWDGE on sync.
    xb = []
    xsrcs = []
    for b in range(BC):
        xt_ = xpool.tile([128, DC, N], BF16, tag="x", name=f"x{b}", bufs=BC)
        xb.append(xt_)
        xsrcs.append(x[b].rearrange("(cc p) n -> p cc n", p=128))
    h0 = slice(0, 512)
    nc.gpsimd.dma_start(xb[0][:, :, h0], xsrcs[0][:, :, h0])
    identb = const.tile([128, 128], BF16, tag="identb")
    make_identity(nc, identb)
    h1 = slice(512, 1024)
    nc.gpsimd.dma_start(xb[0][:, :, h1], xsrcs[0][:, :, h1])
    for b in range(1, BC):
        for h in range(2):
            ns = slice(h * 512, (h + 1) * 512)
            nc.gpsimd.dma_start(xb[b][:, :, ns], xsrcs[b][:, :, ns])

    wn = const.tile([K, D], F32, tag="wn")
    nc.sync.dma_start(wn, w)
    wnb = const.tile([K, D], BF16, tag="wnb")
    nc.vector.tensor_copy(wnb, wn)
    cnat = const.tile([128, DC, K], F32, tag="cnat")
    nc.sync.dma_start(cnat, c.rearrange("(cc p) k -> p cc k", p=128))
    cnb = const.tile([128, DC, K], BF16, tag="cnb")
    nc.vector.tensor_copy(cnb, cnat)

    # conv_w^T in bf16: wTb [128(d), 4, 64(k)]
    wT_ps = ps_xt.tile([128, DC, K], BF16, tag="xt", name="wT_ps")
    for cc in range(DC):
        nc.tensor.transpose(
            wT_ps[:, cc, :], wnb[:, cc * 128:(cc + 1) * 128], identb[:K, :K]
        )
    wTb = const.tile([128, DC, K], BF16, tag="wTb")
    nc.vector.tensor_copy(wTb, wT_ps)
    onesb = const.tile([128, 2], BF16, tag="onesb")
    nc.vector.memset(onesb, 1.0)

    # centers^T replicated on both partition halves: cT2 [128(k2), 512(d)]
    cT2_ps = ps_o.tile([128, DC, 128], F32, tag="o", name="cT2_ps")
    for half in range(2):
        for cc in range(DC):
            nc.tensor.matmul(
                cT2_ps[64 * half:64 * half + 64, cc, :],
                lhsT=cnb[:, cc, :],
                rhs=identb,
            )
    cT2 = const.tile([128, DC, 128], F32, tag="cT2")
    nc.scalar.copy(cT2, cT2_ps)
    cT2f = cT2.rearrange("p cc d -> p (cc d)")

    # assign row-sum accumulators for all 4 batches in one PSUM bank:
    # batch b -> partitions 64*(b%2).., cols 2*(b//2)..
    as_t = ps_as.tile([128, 2 * (BC // 2)], F32, tag="as", name="as_t")

    desc_v = out.rearrange(
        "(bp b2) (cc p k) -> p cc bp b2 k", b2=2, cc=DC, p=128, k=K
    )

    # ---- per batch, software-pipelined across half-batches -------------
    state = {}          # b -> (v2_ps, Vpair, sspair, AN)
    pending = []        # [(b, h, [xs_flat x4])] vlad groups not yet emitted
    epi = []            # batches whose per-batch epilogue is not yet emitted

    def emit_pending():
        while pending:
            pb, ph, xss = pending.pop(0)
            v2_ps_p, _, _, AN_p = state[pb]
            pbase = 64 * (pb % 2)
            pbp = pb // 2
            for jj, xs_flat in enumerate(xss):
                j = NHJ * ph + jj
                nc.tensor.matmul(
                    v2_ps_p[pbase:pbase + 64, :],
                    lhsT=AN_p[:, j, :],
                    rhs=xs_flat,
                    start=(j == 0),
                    stop=(j == NB - 1),
                )
                nc.tensor.matmul(
                    as_t[pbase:pbase + 64, 2 * pbp:2 * pbp + 2],
                    lhsT=AN_p[:, j, :],
                    rhs=onesb,
                    start=(j == 0),
                    stop=(j == NB - 1),
                )

    def emit_batch_epilogue(b):
        v2_ps_b, Vpair_b, sspair_b, _ = state[b]
        base = 64 * (b % 2)
        bp = b // 2
        asum = sp.tile([128, 1], F32, tag="asum", name=f"asum{b}")
        nc.scalar.mul(
            asum[base:base + 64], as_t[base:base + 64, 2 * bp:2 * bp + 1],
            -1.0,
        )
        nc.vector.scalar_tensor_tensor(
            Vpair_b[base:base + 64, :], cT2f[base:base + 64, :],
            asum[base:base + 64], v2_ps_b[base:base + 64, :],
            op0=mybir.AluOpType.mult, op1=mybir.AluOpType.add,
        )
        sq = vp.tile([128, 512], F32, tag="sq", name=f"sq{b}")
        nc.scalar.activation(
            sq[base:base + 64, :], Vpair_b[base:base + 64, :], func=Square,
            accum_out=sspair_b[base:base + 64],
        )

    def emit_pair_epilogue(bp):
        _, Vpair_b, sspair_b, _ = state[2 * bp]
        nrm = sp.tile([128, 1], F32, tag="nrm", name=f"nrm{bp}")
        nc.scalar.sqrt(nrm, sspair_b)
        nrmc = sp.tile([128, 1], F32, tag="nrmc", name=f"nrmc{bp}")
        nc.vector.tensor_scalar_max(nrmc, nrm, EPS)
        rinv = sp.tile([128, 1], F32, tag="rinv", name=f"rinv{bp}")
        nc.vector.reciprocal(rinv, nrmc)
        Vn = vp.tile([128, 512], BF16, tag="Vn", name=f"Vn{bp}")
        nc.vector.tensor_scalar(
            Vn, Vpair_b, rinv, 1.0 / 8.0,
            op0=mybir.AluOpType.mult, op1=mybir.AluOpType.mult,
        )
        # [k2, d] -> [d, k2] via regular identity matmuls (stay warm)
        o_ps = ps_o.tile([128, DC, 128], F32, tag="o", name=f"o{bp}")
        for cc in range(DC):
            nc.tensor.matmul(
                o_ps[:, cc, :],
                lhsT=Vn[:, cc * 128:(cc + 1) * 128],
                rhs=identb,
            )
        o_sb = op.tile([128, DC, 128], F32, tag="osb", name=f"osb{bp}")
        nc.scalar.copy(o_sb, o_ps)
        for b2o in range(2):
            nc.sync.dma_start(
                desc_v[:, :, bp, b2o, :],
                o_sb[:, :, b2o * K:(b2o + 1) * K],
            )

    for b in range(BC):
        bp, b2 = b // 2, b % 2
        if b2 == 0:
            v2_ps = ps_v.tile([128, 512], F32, tag="v", name=f"v{bp}")
            Vpair = vp.tile([128, 512], F32, tag="V", name=f"V{bp}")
            sspair = sp.tile([128, 1], F32, tag="ss", name=f"ss{bp}")
        else:
            v2_ps, Vpair, sspair, _ = state[b - 1]

        AN = atp.tile([128, NB, K], BF16, tag="AN", name=f"AN{b}")
        state[b] = (v2_ps, Vpair, sspair, AN)
        sT = ps_st.tile([128, NB, K], F32, tag="st", name=f"sT{b}")
        AT = atp.tile([128, NB, K], BF16, tag="AT", name=f"AT{b}")
        red = sp.tile([128, NB], F32, tag="red", name=f"red{b}")
        rec = sp.tile([128, NB], F32, tag="rec", name=f"rec{b}")

        for h in range(2):
            hs = slice(NHJ * h, NHJ * h + NHJ)
            xss = []
            for j in range(NHJ * h, NHJ * h + NHJ):
                xt_ps = ps_xt.tile(
                    [128, DC, 128], F32, tag="xt", name=f"xt{b}_{j}"
                )
                for cc in range(DC):
                    xchunk = xb[b][:, cc, j * 128:(j + 1) * 128]
                    # scoresT [n,k] accumulated over d chunks
                    nc.tensor.matmul(
                        sT[:, j, :],
                        lhsT=xchunk,
                        rhs=wTb[:, cc, :],
                        start=(cc == 0),
                        stop=(cc == DC - 1),
                    )
                    # xT [n,d]: regular identity matmul off the same
                    # stationary (f32 PSUM)
                    nc.tensor.matmul(
                        xt_ps[:, cc, :], lhsT=xchunk, rhs=identb,
                        start=True, stop=True,
                    )
                # xT to SBUF promptly (frees the PSUM bank): plain bf16
                # copy, alternating DVE/ACT
                xsT = xst.tile(
                    [128, DC, 128], BF16, tag="xs", name=f"xs{b}_{j}", bufs=8
                )
                xs_flat = xsT.rearrange("p cc d -> p (cc d)")
                xt_flat = xt_ps.rearrange("p cc d -> p (cc d)")
                if j % 2 == 0:
                    nc.vector.tensor_copy(xs_flat, xt_flat)
                else:
                    nc.scalar.copy(xs_flat, xt_flat)
                xss.append(xs_flat)

            # softmax for this half (no max-subtraction: scores ~N(0,1)
            # since conv_w is scaled 1/sqrt(D); exp cannot overflow)
            nc.scalar.activation(AT[:, hs, :], sT[:, hs, :], func=Exp)
            nc.vector.tensor_reduce(
                red[:, hs], AT[:, hs, :], axis=mybir.AxisListType.X,
                op=mybir.AluOpType.add,
            )
            nc.vector.reciprocal(rec[:, hs], red[:, hs])
            rh = rec[:, hs]
            rec_bb = bass.AP(
                tensor=rh.tensor, offset=rh.offset,
                ap=[rh.ap[0], rh.ap[1], [0, K]],
            )
            nc.vector.tensor_mul(AN[:, hs, :], AT[:, hs, :], rec_bb)

            # vlad/asum for the PREVIOUS half-batch: their softmax chain
            # completed while this half's matmuls streamed
            emit_pending()
            pending.append((b, h, xss))

            # epilogues land here too, off the vlad critical path
            while epi:
                emit_batch_epilogue(epi.pop(0))

        epi.append(b)
        if b2 == 1:
            # previous pair's scalar chain + store, before the last halves
            if bp > 0:
                emit_pair_epilogue(bp - 1)

    emit_pending()
    while epi:
        emit_batch_epilogue(epi.pop(0))
    emit_pair_epilogue(BC // 2 - 1)


_NC_CACHE = None


def _build_nc():
    global _NC_CACHE
    if _NC_CACHE is not None:
        return _NC_CACHE
    from contextlib import ExitStack

    nc = bacc.Bacc("TRN2", target_bir_lowering=False, debug=False,
                   num_devices=NCORES)
    x = nc.dram_tensor("x", [BC, D, N], F32, kind="ExternalInput").ap()
    w = nc.dram_tensor("conv_w", [K, D], F32, kind="ExternalInput").ap()
    c = nc.dram_tensor("centers", [D, K], F32, kind="ExternalInput").ap()
    out = nc.dram_tensor("desc", [BC, D * K], F32, kind="ExternalOutput").ap()
    with tile.TileContext(nc) as tc, ExitStack() as ctx:
        _netvlad_core(ctx, tc, out, x, w, c)
    nc.compile()
    _NC_CACHE = nc
    return nc


def kernel(x, conv_w, centers):
    x = np.ascontiguousarray(x, dtype=np.float32)
    conv_w = np.ascontiguousarray(conv_w, dtype=np.float32)
    centers = np.ascontiguousarray(centers, dtype=np.float32)
    nc = _build_nc()
    in_maps = [
        {
            "x": np.ascontiguousarray(x[i * BC:(i + 1) * BC]),
            "conv_w": conv_w,
            "centers": centers,
        }
        for i in range(NCORES)
    ]
    res = run_bass_kernel_spmd(nc, in_maps, core_ids=list(range(NCORES)))
    return np.concatenate([r["desc"] for r in res.results], axis=0)


# revision 22
# speedup vs baseline: 1.4142x; 1.0526x over previous
"""NetVLAD pooling kernel for Trainium2 (Bass/Tile), 8-core data-parallel.

Reference computation (per batch b):
    scores = conv_w @ x[b]                  # [K, N]
    assign = softmax(scores, axis=K)
    vlad   = x[b] @ assign.T - centers * assign.sum(n)   # [D, K]
    vlad  /= max(||vlad||_2 over D, eps)    # intra-norm per cluster column
    desc   = vlad.reshape(D*K) / max(||.||_2, eps)

Shapes: x [32, 512, 1024] f32, conv_w [64, 512], centers [512, 64],
output desc [32, 32768] f32.  Sharding: data-parallel over batch,
4 batches per core; params replicated.

v8 design notes.  The kernel is PE-bound; in this toolchain every
matmul self-loads its stationary operand (~115ns serial LDWEIGHTS,
no dedup) and transpose-mode matmuls run on a cold clock (~250ns), so
the structure minimizes PE instruction count and keeps the hot loop on
regular (HAM-warming) matmuls:

  * x casts f32->bf16 *during* the DMA (SWDGE, gpsimd) in 8 half-batch
    chunks; batch 0's first half leads the Q7 queue; conv_w/centers ride
    HWDGE on the idle sync engine.
  * scoresT [n,k] come from x-chunk-stationary matmuls streaming
    conv_w^T (the same stationary then streams the identity to emit
    xT [n,d]) - softmax over k is then a free-dim DVE reduce, no E
    transposes, and x is never re-loaded from another layout.
  * the xT identity-matmuls are REGULAR matmuls into f32 PSUM; the
    PSUM->SBUF bf16 copies alternate DVE/ACT per chunk.
  * softmax chain runs per half-batch (8 wide ops, not 32 narrow ones);
    normalization folds into AN = exp*rec, so copies stay plain and the
    assign row-sums are ones-matmuls off the same AN stationaries.
  * vlad/asum matmuls are emitted one half-batch late (PE queue is
    strict FIFO): the next half's score/transpose matmuls sit between
    them and the softmax chain they depend on, so the PE never stalls.
  * batches pair into PSUM halves ([0:64] even, [64:128] odd, matmul
    column tiling); the epilogue's wide ops run per batch as soon as its
    vlad closes, the cheap scalar chain runs once per pair (single sqrt
    visit per pair - the ACT table switch exp<->sqrt costs 1.3us), and
    the final [k,d]->[d,k] flip uses regular identity matmuls.
  * the second L2 normalization folds to 1/8 (the K=64 unit columns give
    ||desc|| = 8 exactly).

bf16 rounding of x/w/assign contributes ~2e-3 relative error, well
inside the 2e-2 gate (measured: see test.py output).
"""

import numpy as np

import concourse.bass as bass
from concourse import bacc
import concourse.mybir as mybir
import concourse.tile as tile
from concourse.bass_utils import run_bass_kernel_spmd
from concourse.masks import make_identity

B, D, K, N = 32, 512, 64, 1024
NCORES = 8
BC = B // NCORES          # batches per core
F32 = mybir.dt.float32
BF16 = mybir.dt.bfloat16
EPS = 1e-12

DC = D // 128             # d chunks (4)
NB = N // 128             # n chunks per batch (8)
NHJ = NB // 2             # n chunks per half (4)


def _netvlad_core(ctx, tc, out, x, w, c):
    """Emit the per-core tile program.

    out: desc [BC, D*K] f32 DRAM     x: [BC, D, N] f32 DRAM
    w:   conv_w [K, D] f32 DRAM      c: centers [D, K] f32 DRAM
    """
    nc = tc.nc
    Exp = mybir.ActivationFunctionType.Exp
    Square = mybir.ActivationFunctionType.Square

    const = ctx.enter_context(tc.tile_pool(name="const", bufs=1))
    xpool = ctx.enter_context(tc.tile_pool(name="xp", bufs=1))
    atp = ctx.enter_context(tc.tile_pool(name="atp", bufs=2))
    sp = ctx.enter_context(tc.tile_pool(name="sp", bufs=2))
    xst = ctx.enter_context(tc.tile_pool(name="xst", bufs=8))
    vp = ctx.enter_context(tc.tile_pool(name="vp", bufs=2))
    op = ctx.enter_context(tc.tile_pool(name="op", bufs=2))
    # PSUM: st(1) + xt(3) + v(2) + o(1) + as(1) = 8 banks
    ps_st = ctx.enter_context(tc.tile_pool(name="ps_st", bufs=1, space="PSUM"))
    ps_xt = ctx.enter_context(tc.tile_pool(name="ps_xt", bufs=3, space="PSUM"))
    ps_v = ctx.enter_context(tc.tile_pool(name="ps_v", bufs=2, space="PSUM"))
    ps_o = ctx.enter_context(tc.tile_pool(name="ps_o", bufs=1, space="PSUM"))
    ps_as = ctx.enter_context(tc.tile_pool(name="ps_as", bufs=1, space="PSUM"))

    # ---- startup-critical ordering.  SWDGE descriptor emission is ~3us
    # per half-batch and strictly serial on Q7, so batch 0's first half
    # leads; identity build interleaves; w/c ride HWDGE on sync.
    xb = []
    xsrcs = []
    for b in range(BC):
        xt_ = xpool.tile([128, DC, N], BF16, tag="x", name=f"x{b}", bufs=BC)
        xb.append(xt_)
        xsrcs.append(x[b].rearrange("(cc p) n -> p cc n", p=128))
    # batch 0's first half rides HWDGE (fast issue, no Q7 emission wait)
    # as f32 with a DVE cast; everything else streams via SWDGE cast-DMA
    h0 = slice(0, 512)
    x0f = const.tile([128, DC, 512], F32, tag="x0f")
    nc.sync.dma_start(x0f, xsrcs[0][:, :, h0])
    nc.vector.tensor_copy(xb[0][:, :, h0], x0f)
    identb = const.tile([128, 128], BF16, tag="identb")
    make_identity(nc, identb)
    h1 = slice(512, 1024)
    nc.gpsimd.dma_start(xb[0][:, :, h1], xsrcs[0][:, :, h1])
    for b in range(1, BC):
        for h in range(2):
            ns = slice(h * 512, (h + 1) * 512)
            nc.gpsimd.dma_start(xb[b][:, :, ns], xsrcs[b][:, :, ns])

    wn = const.tile([K, D], F32, tag="wn")
    nc.sync.dma_start(wn, w)
    wnb = const.tile([K, D], BF16, tag="wnb")
    nc.vector.tensor_copy(wnb, wn)
    cnat = const.tile([128, DC, K], F32, tag="cnat")
    nc.sync.dma_start(cnat, c.rearrange("(cc p) k -> p cc k", p=128))
    cnb = const.tile([128, DC, K], BF16, tag="cnb")
    nc.vector.tensor_copy(cnb, cnat)

    # conv_w^T in bf16: wTb [128(d), 4, 64(k)]
    wT_ps = ps_xt.tile([128, DC, K], BF16, tag="xt", name="wT_ps")
    for cc in range(DC):
        nc.tensor.transpose(
            wT_ps[:, cc, :], wnb[:, cc * 128:(cc + 1) * 128], identb[:K, :K]
        )
    wTb = const.tile([128, DC, K], BF16, tag="wTb")
    nc.vector.tensor_copy(wTb, wT_ps)
    onesb = const.tile([128, 2], BF16, tag="onesb")
    nc.vector.memset(onesb, 1.0)

    # centers^T replicated on both partition halves: cT2 [128(k2), 512(d)]
    cT2_ps = ps_o.tile([128, DC, 128], F32, tag="o", name="cT2_ps")
    for half in range(2):
        for cc in range(DC):
            nc.tensor.matmul(
                cT2_ps[64 * half:64 * half + 64, cc, :],
                lhsT=cnb[:, cc, :],
                rhs=identb,
            )
    cT2 = const.tile([128, DC, 128], F32, tag="cT2")
    nc.scalar.copy(cT2, cT2_ps)
    cT2f = cT2.rearrange("p cc d -> p (cc d)")

    # assign row-sum accumulators for all 4 batches in one PSUM bank:
    # batch b -> partitions 64*(b%2).., cols 2*(b//2)..
    as_t = ps_as.tile([128, 2 * (BC // 2)], F32, tag="as", name="as_t")

    desc_v = out.rearrange(
        "(bp b2) (cc p k) -> p cc bp b2 k", b2=2, cc=DC, p=128, k=K
    )

    # ---- per batch, software-pipelined across half-batches -------------
    state = {}          # b -> (v2_ps, Vpair, sspair, AN)
    pending = []        # [(b, h, [xs_flat x4])] vlad groups not yet emitted
    epi = []            # batches whose per-batch epilogue is not yet emitted

    def emit_pending():
        while pending:
            pb, ph, xss = pending.pop(0)
            v2_ps_p, _, _, AN_p = state[pb]
            pbase = 64 * (pb % 2)
            pbp = pb // 2
            for jj, xs_flat in enumerate(xss):
                j = NHJ * ph + jj
                nc.tensor.matmul(
                    v2_ps_p[pbase:pbase + 64, :],
                    lhsT=AN_p[:, j, :],
                    rhs=xs_flat,
                    start=(j == 0),
                    stop=(j == NB - 1),
                )
                nc.tensor.matmul(
                    as_t[pbase:pbase + 64, 2 * pbp:2 * pbp + 2],
                    lhsT=AN_p[:, j, :],
                    rhs=onesb,
                    start=(j == 0),
                    stop=(j == NB - 1),
                )

    def emit_batch_epilogue(b):
        v2_ps_b, Vpair_b, sspair_b, _ = state[b]
        base = 64 * (b % 2)
        bp = b // 2
        asum = sp.tile([128, 1], F32, tag="asum", name=f"asum{b}")
        nc.scalar.mul(
            asum[base:base + 64], as_t[base:base + 64, 2 * bp:2 * bp + 1],
            -1.0,
        )
        nc.vector.scalar_tensor_tensor(
            Vpair_b[base:base + 64, :], cT2f[base:base + 64, :],
            asum[base:base + 64], v2_ps_b[base:base + 64, :],
            op0=mybir.AluOpType.mult, op1=mybir.AluOpType.add,
        )
        sq = vp.tile([128, 512], F32, tag="sq", name=f"sq{b}")
        nc.scalar.activation(
            sq[base:base + 64, :], Vpair_b[base:base + 64, :], func=Square,
            accum_out=sspair_b[base:base + 64],
        )

    def emit_pair_epilogue(bp):
        _, Vpair_b, sspair_b, _ = state[2 * bp]
        nrm = sp.tile([128, 1], F32, tag="nrm", name=f"nrm{bp}")
        nc.scalar.sqrt(nrm, sspair_b)
        nrmc = sp.tile([128, 1], F32, tag="nrmc", name=f"nrmc{bp}")
        nc.vector.tensor_scalar_max(nrmc, nrm, EPS)
        rinv = sp.tile([128, 1], F32, tag="rinv", name=f"rinv{bp}")
        nc.vector.reciprocal(rinv, nrmc)
        Vn = vp.tile([128, 512], BF16, tag="Vn", name=f"Vn{bp}")
        nc.vector.tensor_scalar(
            Vn, Vpair_b, rinv, 1.0 / 8.0,
            op0=mybir.AluOpType.mult, op1=mybir.AluOpType.mult,
        )
        # [k2, d] -> [d, k2] via regular identity matmuls (stay warm)
        o_ps = ps_o.tile([128, DC, 128], F32, tag="o", name=f"o{bp}")
        for cc in range(DC):
            nc.tensor.matmul(
                o_ps[:, cc, :],
                lhsT=Vn[:, cc * 128:(cc + 1) * 128],
                rhs=identb,
            )
        o_sb = op.tile([128, DC, 128], F32, tag="osb", name=f"osb{bp}")
        nc.scalar.copy(o_sb, o_ps)
        for b2o in range(2):
            nc.sync.dma_start(
                desc_v[:, :, bp, b2o, :],
                o_sb[:, :, b2o * K:(b2o + 1) * K],
            )

    for b in range(BC):
        bp, b2 = b // 2, b % 2
        if b2 == 0:
            v2_ps = ps_v.tile([128, 512], F32, tag="v", name=f"v{bp}")
            Vpair = vp.tile([128, 512], F32, tag="V", name=f"V{bp}")
            sspair = sp.tile([128, 1], F32, tag="ss", name=f"ss{bp}")
        else:
            v2_ps, Vpair, sspair, _ = state[b - 1]

        AN = atp.tile([128, NB, K], BF16, tag="AN", name=f"AN{b}")
        state[b] = (v2_ps, Vpair, sspair, AN)
        sT = ps_st.tile([128, NB, K], F32, tag="st", name=f"sT{b}")
        AT = atp.tile([128, NB, K], BF16, tag="AT", name=f"AT{b}")
        red = sp.tile([128, NB], F32, tag="red", name=f"red{b}")
        rec = sp.tile([128, NB], F32, tag="rec", name=f"rec{b}")

        for h in range(2):
            hs = slice(NHJ * h, NHJ * h + NHJ)
            xss = []
            for j in range(NHJ * h, NHJ * h + NHJ):
                xt_ps = ps_xt.tile(
                    [128, DC, 128], F32, tag="xt", name=f"xt{b}_{j}"
                )
                for cc in range(DC):
                    xchunk = xb[b][:, cc, j * 128:(j + 1) * 128]
                    # scoresT [n,k] accumulated over d chunks
                    nc.tensor.matmul(
                        sT[:, j, :],
                        lhsT=xchunk,
                        rhs=wTb[:, cc, :],
                        start=(cc == 0),
                        stop=(cc == DC - 1),
                    )
                    # xT [n,d]: regular identity matmul off the same
                    # stationary (f32 PSUM)
                    nc.tensor.matmul(
                        xt_ps[:, cc, :], lhsT=xchunk, rhs=identb,
                        start=True, stop=True,
                    )
                # xT to SBUF promptly (frees the PSUM bank): plain bf16
                # copy, alternating DVE/ACT
                xsT = xst.tile(
                    [128, DC, 128], BF16, tag="xs", name=f"xs{b}_{j}", bufs=8
                )
                xs_flat = xsT.rearrange("p cc d -> p (cc d)")
                xt_flat = xt_ps.rearrange("p cc d -> p (cc d)")
                if j % 2 == 0:
                    nc.vector.tensor_copy(xs_flat, xt_flat)
                else:
                    nc.scalar.copy(xs_flat, xt_flat)
                xss.append(xs_flat)

            # softmax for this half (no max-subtraction: scores ~N(0,1)
            # since conv_w is scaled 1/sqrt(D); exp cannot overflow)
            nc.scalar.activation(AT[:, hs, :], sT[:, hs, :], func=Exp)
            nc.vector.tensor_reduce(
                red[:, hs], AT[:, hs, :], axis=mybir.AxisListType.X,
                op=mybir.AluOpType.add,
            )
            nc.vector.reciprocal(rec[:, hs], red[:, hs])
            rh = rec[:, hs]
            rec_bb = bass.AP(
                tensor=rh.tensor, offset=rh.offset,
                ap=[rh.ap[0], rh.ap[1], [0, K]],
            )
            nc.vector.tensor_mul(AN[:, hs, :], AT[:, hs, :], rec_bb)

            # vlad/asum for the PREVIOUS half-batch: their softmax chain
            # completed while this half's matmuls streamed
            emit_pending()
            pending.append((b, h, xss))

            # epilogues land here too, off the vlad critical path
            while epi:
                emit_batch_epilogue(epi.pop(0))

        epi.append(b)
        if b2 == 1:
            # previous pair's scalar chain + store, before the last halves
            if bp > 0:
                emit_pair_epilogue(bp - 1)

    emit_pending()
    while epi:
        emit_batch_epilogue(epi.pop(0))
    emit_pair_epilogue(BC // 2 - 1)


_NC_CACHE = None


def _build_nc():
    global _NC_CACHE
    if _NC_CACHE is not None:
        return _NC_CACHE
    from contextlib import ExitStack

    nc = bacc.Bacc("TRN2", target_bir_lowering=False, debug=False,
                   num_devices=NCORES)
    x = nc.dram_tensor("x", [BC, D, N], F32, kind="ExternalInput").ap()
    w = nc.dram_tensor("conv_w", [K, D], F32, kind="ExternalInput").ap()
    c = nc.dram_tensor("centers", [D, K], F32, kind="ExternalInput").ap()
    out = nc.dram_tensor("desc", [BC, D * K], F32, kind="ExternalOutput").ap()
    with tile.TileContext(nc) as tc, ExitStack() as ctx:
        _netvlad_core(ctx, tc, out, x, w, c)
    nc.compile()
    _NC_CACHE = nc
    return nc


def kernel(x, conv_w, centers):
    x = np.ascontiguousarray(x, dtype=np.float32)
    conv_w = np.ascontiguousarray(conv_w, dtype=np.float32)
    centers = np.ascontiguousarray(centers, dtype=np.float32)
    nc = _build_nc()
    in_maps = [
        {
            "x": np.ascontiguousarray(x[i * BC:(i + 1) * BC]),
            "conv_w": conv_w,
            "centers": centers,
        }
        for i in range(NCORES)
    ]
    res = run_bass_kernel_spmd(nc, in_maps, core_ids=list(range(NCORES)))
    return np.concatenate([r["desc"] for r in res.results], axis=0)


# revision 26
# speedup vs baseline: 1.4326x; 1.0131x over previous
"""NetVLAD pooling kernel for Trainium2 (Bass/Tile), 8-core data-parallel.

Reference computation (per batch b):
    scores = conv_w @ x[b]                  # [K, N]
    assign = softmax(scores, axis=K)
    vlad   = x[b] @ assign.T - centers * assign.sum(n)   # [D, K]
    vlad  /= max(||vlad||_2 over D, eps)    # intra-norm per cluster column
    desc   = vlad.reshape(D*K) / max(||.||_2, eps)

Shapes: x [32, 512, 1024] f32, conv_w [64, 512], centers [512, 64],
output desc [32, 32768] f32.  Sharding: data-parallel over batch,
4 batches per core; params replicated.

v8 design notes.  The kernel is PE-bound; in this toolchain every
matmul self-loads its stationary operand (~115ns serial LDWEIGHTS,
no dedup) and transpose-mode matmuls run on a cold clock (~250ns), so
the structure minimizes PE instruction count and keeps the hot loop on
regular (HAM-warming) matmuls:

  * x casts f32->bf16 *during* the DMA (SWDGE, gpsimd) in 8 half-batch
    chunks; batch 0's first half leads the Q7 queue; conv_w/centers ride
    HWDGE on the idle sync engine.
  * scoresT [n,k] come from x-chunk-stationary matmuls streaming
    conv_w^T (the same stationary then streams the identity to emit
    xT [n,d]) - softmax over k is then a free-dim DVE reduce, no E
    transposes, and x is never re-loaded from another layout.
  * the xT identity-matmuls are REGULAR matmuls into f32 PSUM; the
    PSUM->SBUF bf16 copies alternate DVE/ACT per chunk.
  * softmax chain runs per half-batch (8 wide ops, not 32 narrow ones);
    normalization folds into AN = exp*rec, so copies stay plain and the
    assign row-sums are ones-matmuls off the same AN stationaries.
  * vlad/asum matmuls are emitted one half-batch late (PE queue is
    strict FIFO): the next half's score/transpose matmuls sit between
    them and the softmax chain they depend on, so the PE never stalls.
  * batches pair into PSUM halves ([0:64] even, [64:128] odd, matmul
    column tiling); the epilogue's wide ops run per batch as soon as its
    vlad closes, the cheap scalar chain runs once per pair (single sqrt
    visit per pair - the ACT table switch exp<->sqrt costs 1.3us), and
    the final [k,d]->[d,k] flip uses regular identity matmuls.
  * the second L2 normalization folds to 1/8 (the K=64 unit columns give
    ||desc|| = 8 exactly).

bf16 rounding of x/w/assign contributes ~2e-3 relative error, well
inside the 2e-2 gate (measured: see test.py output).
"""

import numpy as np

import concourse.bass as bass
from concourse import bacc
import concourse.mybir as mybir
import concourse.tile as tile
from concourse.bass_utils import run_bass_kernel_spmd
from concourse.masks import make_identity

B, D, K, N = 32, 512, 64, 1024
NCORES = 8
BC = B // NCORES          # batches per core
F32 = mybir.dt.float32
BF16 = mybir.dt.bfloat16
EPS = 1e-12

DC = D // 128             # d chunks (4)
NB = N // 128             # n chunks per batch (8)
NHJ = NB // 2             # n chunks per half (4)


def _netvlad_core(ctx, tc, out, x, w, c):
    """Emit the per-core tile program.

    out: desc [BC, D*K] f32 DRAM     x: [BC, D, N] f32 DRAM
    w:   conv_w [K, D] f32 DRAM      c: centers [D, K] f32 DRAM
    """
    nc = tc.nc
    Exp = mybir.ActivationFunctionType.Exp
    Square = mybir.ActivationFunctionType.Square

    const = ctx.enter_context(tc.tile_pool(name="const", bufs=1))
    xpool = ctx.enter_context(tc.tile_pool(name="xp", bufs=1))
    atp = ctx.enter_context(tc.tile_pool(name="atp", bufs=2))
    sp = ctx.enter_context(tc.tile_pool(name="sp", bufs=2))
    xst = ctx.enter_context(tc.tile_pool(name="xst", bufs=8))
    vp = ctx.enter_context(tc.tile_pool(name="vp", bufs=2))
    op = ctx.enter_context(tc.tile_pool(name="op", bufs=2))
    # PSUM: st(1) + xt(3) + v(2) + o(1) + as(1) = 8 banks
    ps_st = ctx.enter_context(tc.tile_pool(name="ps_st", bufs=1, space="PSUM"))
    ps_xt = ctx.enter_context(tc.tile_pool(name="ps_xt", bufs=3, space="PSUM"))
    ps_v = ctx.enter_context(tc.tile_pool(name="ps_v", bufs=2, space="PSUM"))
    ps_o = ctx.enter_context(tc.tile_pool(name="ps_o", bufs=1, space="PSUM"))
    ps_as = ctx.enter_context(tc.tile_pool(name="ps_as", bufs=1, space="PSUM"))

    # ---- startup-critical ordering.  SWDGE descriptor emission is ~3us
    # per half-batch and strictly serial on Q7, so batch 0's first half
    # leads; identity build interleaves; w/c ride HWDGE on sync.
    xb = []
    xsrcs = []
    for b in range(BC):
        xt_ = xpool.tile([128, DC, N], BF16, tag="x", name=f"x{b}", bufs=BC)
        xb.append(xt_)
        xsrcs.append(x[b].rearrange("(cc p) n -> p cc n", p=128))
    # batch 0's first half rides HWDGE (fast issue, no Q7 emission wait)
    # as f32 with a DVE cast; everything else streams via SWDGE cast-DMA
    h0 = slice(0, 512)
    x0f = const.tile([128, DC, 512], F32, tag="x0f")
    nc.sync.dma_start(x0f, xsrcs[0][:, :, h0])
    nc.vector.tensor_copy(xb[0][:, :, h0], x0f)
    identb = const.tile([128, 128], BF16, tag="identb")
    make_identity(nc, identb)
    h1 = slice(512, 1024)
    nc.gpsimd.dma_start(xb[0][:, :, h1], xsrcs[0][:, :, h1])
    for b in range(1, BC):
        for h in range(2):
            ns = slice(h * 512, (h + 1) * 512)
            nc.gpsimd.dma_start(xb[b][:, :, ns], xsrcs[b][:, :, ns])

    wn = const.tile([K, D], F32, tag="wn")
    nc.sync.dma_start(wn, w)
    wnb = const.tile([K, D], BF16, tag="wnb")
    nc.vector.tensor_copy(wnb, wn)
    cnat = const.tile([128, DC, K], F32, tag="cnat")
    nc.sync.dma_start(cnat, c.rearrange("(cc p) k -> p cc k", p=128))
    cnb = const.tile([128, DC, K], BF16, tag="cnb")
    nc.vector.tensor_copy(cnb, cnat)

    # conv_w^T in bf16: wTb [128(d), 4, 64(k)]
    wT_ps = ps_xt.tile([128, DC, K], BF16, tag="xt", name="wT_ps")
    for cc in range(DC):
        nc.tensor.transpose(
            wT_ps[:, cc, :], wnb[:, cc * 128:(cc + 1) * 128], identb[:K, :K]
        )
    wTb = const.tile([128, DC, K], BF16, tag="wTb")
    nc.vector.tensor_copy(wTb, wT_ps)
    onesb = const.tile([128, 2], BF16, tag="onesb")
    nc.vector.memset(onesb, 1.0)

    # centers^T replicated on both partition halves: cT2 [128(k2), 512(d)]
    cT2_ps = ps_o.tile([128, DC, 128], F32, tag="o", name="cT2_ps")
    for half in range(2):
        for cc in range(DC):
            nc.tensor.matmul(
                cT2_ps[64 * half:64 * half + 64, cc, :],
                lhsT=cnb[:, cc, :],
                rhs=identb,
            )
    cT2 = const.tile([128, DC, 128], F32, tag="cT2")
    nc.scalar.copy(cT2, cT2_ps)
    cT2f = cT2.rearrange("p cc d -> p (cc d)")

    # assign row-sum accumulators for all 4 batches in one PSUM bank:
    # batch b -> partitions 64*(b%2).., cols 2*(b//2)..
    as_t = ps_as.tile([128, 2 * (BC // 2)], F32, tag="as", name="as_t")

    desc_v = out.rearrange(
        "(bp b2) (cc p k) -> p cc bp b2 k", b2=2, cc=DC, p=128, k=K
    )

    # ---- per batch, software-pipelined across half-batches -------------
    state = {}          # b -> (v2_ps, Vpair, sspair, AN)
    pending = []        # [(b, h, [xs_flat x4])] vlad groups not yet emitted
    epi = []            # batches whose per-batch epilogue is not yet emitted

    def emit_pending():
        while pending:
            pb, pjs, xss = pending.pop(0)
            v2_ps_p, _, _, AN_p = state[pb]
            pbase = 64 * (pb % 2)
            pbp = pb // 2
            for j, xs_flat in zip(pjs, xss):
                nc.tensor.matmul(
                    v2_ps_p[pbase:pbase + 64, :],
                    lhsT=AN_p[:, j, :],
                    rhs=xs_flat,
                    start=(j == 0),
                    stop=(j == NB - 1),
                )
                nc.tensor.matmul(
                    as_t[pbase:pbase + 64, 2 * pbp:2 * pbp + 2],
                    lhsT=AN_p[:, j, :],
                    rhs=onesb,
                    start=(j == 0),
                    stop=(j == NB - 1),
                )

    def emit_batch_epilogue(b):
        v2_ps_b, Vpair_b, sspair_b, _ = state[b]
        base = 64 * (b % 2)
        bp = b // 2
        asum = sp.tile([128, 1], F32, tag="asum", name=f"asum{b}")
        nc.scalar.mul(
            asum[base:base + 64], as_t[base:base + 64, 2 * bp:2 * bp + 1],
            -1.0,
        )
        nc.vector.scalar_tensor_tensor(
            Vpair_b[base:base + 64, :], cT2f[base:base + 64, :],
            asum[base:base + 64], v2_ps_b[base:base + 64, :],
            op0=mybir.AluOpType.mult, op1=mybir.AluOpType.add,
        )
        sq = vp.tile([128, 512], F32, tag="sq", name=f"sq{b}")
        nc.scalar.activation(
            sq[base:base + 64, :], Vpair_b[base:base + 64, :], func=Square,
            accum_out=sspair_b[base:base + 64],
        )

    def emit_pair_epilogue(bp):
        _, Vpair_b, sspair_b, _ = state[2 * bp]
        nrm = sp.tile([128, 1], F32, tag="nrm", name=f"nrm{bp}")
        nc.scalar.sqrt(nrm, sspair_b)
        nrmc = sp.tile([128, 1], F32, tag="nrmc", name=f"nrmc{bp}")
        nc.vector.tensor_scalar_max(nrmc, nrm, EPS)
        rinv = sp.tile([128, 1], F32, tag="rinv", name=f"rinv{bp}")
        nc.vector.reciprocal(rinv, nrmc)
        Vn = vp.tile([128, 512], BF16, tag="Vn", name=f"Vn{bp}")
        nc.vector.tensor_scalar(
            Vn, Vpair_b, rinv, 1.0 / 8.0,
            op0=mybir.AluOpType.mult, op1=mybir.AluOpType.mult,
        )
        # [k2, d] -> [d, k2] via regular identity matmuls (stay warm)
        o_ps = ps_o.tile([128, DC, 128], F32, tag="o", name=f"o{bp}")
        for cc in range(DC):
            nc.tensor.matmul(
                o_ps[:, cc, :],
                lhsT=Vn[:, cc * 128:(cc + 1) * 128],
                rhs=identb,
            )
        o_sb = op.tile([128, DC, 128], F32, tag="osb", name=f"osb{bp}")
        nc.scalar.copy(o_sb, o_ps)
        for b2o in range(2):
            nc.sync.dma_start(
                desc_v[:, :, bp, b2o, :],
                o_sb[:, :, b2o * K:(b2o + 1) * K],
            )

    for b in range(BC):
        bp, b2 = b // 2, b % 2
        if b2 == 0:
            v2_ps = ps_v.tile([128, 512], F32, tag="v", name=f"v{bp}")
            Vpair = vp.tile([128, 512], F32, tag="V", name=f"V{bp}")
            sspair = sp.tile([128, 1], F32, tag="ss", name=f"ss{bp}")
        else:
            v2_ps, Vpair, sspair, _ = state[b - 1]

        AN = atp.tile([128, NB, K], BF16, tag="AN", name=f"AN{b}")
        state[b] = (v2_ps, Vpair, sspair, AN)
        sT = ps_st.tile([128, NB, K], F32, tag="st", name=f"sT{b}")
        AT = atp.tile([128, NB, K], BF16, tag="AT", name=f"AT{b}")
        red = sp.tile([128, NB], F32, tag="red", name=f"red{b}")
        rec = sp.tile([128, NB], F32, tag="rec", name=f"rec{b}")

        for h in range(2):
            hs = slice(NHJ * h, NHJ * h + NHJ)
            last_half = (b == BC - 1 and h == 1)
            xss = []
            for j in range(NHJ * h, NHJ * h + NHJ):
                xt_ps = ps_xt.tile(
                    [128, DC, 128], F32, tag="xt", name=f"xt{b}_{j}"
                )
                for cc in range(DC):
                    xchunk = xb[b][:, cc, j * 128:(j + 1) * 128]
                    # scoresT [n,k] accumulated over d chunks
                    nc.tensor.matmul(
                        sT[:, j, :],
                        lhsT=xchunk,
                        rhs=wTb[:, cc, :],
                        start=(cc == 0),
                        stop=(cc == DC - 1),
                    )
                    # xT [n,d]: regular identity matmul off the same
                    # stationary (f32 PSUM)
                    nc.tensor.matmul(
                        xt_ps[:, cc, :], lhsT=xchunk, rhs=identb,
                        start=True, stop=True,
                    )
                # xT to SBUF promptly (frees the PSUM bank): plain bf16
                # copy, alternating DVE/ACT
                xsT = xst.tile(
                    [128, DC, 128], BF16, tag="xs", name=f"xs{b}_{j}", bufs=8
                )
                xs_flat = xsT.rearrange("p cc d -> p (cc d)")
                xt_flat = xt_ps.rearrange("p cc d -> p (cc d)")
                if j % 2 == 0:
                    nc.vector.tensor_copy(xs_flat, xt_flat)
                else:
                    nc.scalar.copy(xs_flat, xt_flat)
                xss.append(xs_flat)
                if last_half:
                    # drain the tail at chunk granularity: per-j softmax
                    # chain + immediate vlad, so the kernel end isn't a
                    # serial half-batch chain
                    js = slice(j, j + 1)
                    nc.scalar.activation(AT[:, js, :], sT[:, js, :],
                                         func=Exp)
                    nc.vector.tensor_reduce(
                        red[:, js], AT[:, js, :], axis=mybir.AxisListType.X,
                        op=mybir.AluOpType.add,
                    )
                    nc.vector.reciprocal(rec[:, js], red[:, js])
                    nc.vector.tensor_scalar(
                        AN[:, j, :], AT[:, j, :], rec[:, j:j + 1], None,
                        op0=mybir.AluOpType.mult,
                    )
                    if j == NHJ * h:
                        emit_pending()
                    pending.append((b, [j], [xs_flat]))
                    emit_pending()

            if not last_half:
                # softmax for this half (no max-subtraction: scores ~N(0,1)
                # since conv_w is scaled 1/sqrt(D); exp cannot overflow)
                nc.scalar.activation(AT[:, hs, :], sT[:, hs, :], func=Exp)
                nc.vector.tensor_reduce(
                    red[:, hs], AT[:, hs, :], axis=mybir.AxisListType.X,
                    op=mybir.AluOpType.add,
                )
                nc.vector.reciprocal(rec[:, hs], red[:, hs])
                rh = rec[:, hs]
                rec_bb = bass.AP(
                    tensor=rh.tensor, offset=rh.offset,
                    ap=[rh.ap[0], rh.ap[1], [0, K]],
                )
                nc.vector.tensor_mul(AN[:, hs, :], AT[:, hs, :], rec_bb)

                # vlad/asum for the PREVIOUS half-batch: their softmax
                # chain completed while this half's matmuls streamed
                emit_pending()
                pending.append(
                    (b, list(range(NHJ * h, NHJ * h + NHJ)), xss)
                )

            # epilogues land here too, off the vlad critical path
            while epi:
                emit_batch_epilogue(epi.pop(0))

        epi.append(b)
        if b2 == 1:
            # previous pair's scalar chain + store, before the last halves
            if bp > 0:
                emit_pair_epilogue(bp - 1)

    emit_pending()
    while epi:
        emit_batch_epilogue(epi.pop(0))
    emit_pair_epilogue(BC // 2 - 1)


_NC_CACHE = None


def _build_nc():
    global _NC_CACHE
    if _NC_CACHE is not None:
        return _NC_CACHE
    from contextlib import ExitStack

    nc = bacc.Bacc("TRN2", target_bir_lowering=False, debug=False,
                   num_devices=NCORES)
    x = nc.dram_tensor("x", [BC, D, N], F32, kind="ExternalInput").ap()
    w = nc.dram_tensor("conv_w", [K, D], F32, kind="ExternalInput").ap()
    c = nc.dram_tensor("centers", [D, K], F32, kind="ExternalInput").ap()
    out = nc.dram_tensor("desc", [BC, D * K], F32, kind="ExternalOutput").ap()
    with tile.TileContext(nc) as tc, ExitStack() as ctx:
        _netvlad_core(ctx, tc, out, x, w, c)
    nc.compile()
    _NC_CACHE = nc
    return nc


def kernel(x, conv_w, centers):
    x = np.ascontiguousarray(x, dtype=np.float32)
    conv_w = np.ascontiguousarray(conv_w, dtype=np.float32)
    centers = np.ascontiguousarray(centers, dtype=np.float32)
    nc = _build_nc()
    in_maps = [
        {
            "x": np.ascontiguousarray(x[i * BC:(i + 1) * BC]),
            "conv_w": conv_w,
            "centers": centers,
        }
        for i in range(NCORES)
    ]
    res = run_bass_kernel_spmd(nc, in_maps, core_ids=list(range(NCORES)))
    return np.concatenate([r["desc"] for r in res.results], axis=0)
